# revision 24
# baseline (speedup 1.0000x reference)
"""Trainium2 Bass kernel for nn_ComplexDotProduct.

  out[b, o, n] = sum_c complex(x)[b, c, n] * complex(w)[o, c, n] + bias[o]
  B=64, C=128, N=1024, O=512.

Strategy
--------
Shard N across the 8 cores (128 positions each) — no tensor is replicated
(x, w and out are all sliced by n), so per-core HBM traffic is the global
minimum for the dtype.

The kernel is DMA-bandwidth-bound (16 SDMA engines x 22.5 B/ns, ~0.88
achieved utilization ~= 315 B/ns per core), so everything moves in bf16:
rel-err tolerance is 2e-2 and bf16 in/out costs ~3e-3. Per-core traffic:
w 33.6 MB + x 4.2 MB + out 16.8 MB ~= 54.8 MB -> ~175 us. The TensorE
work (4 bf16 matmuls x 512 moving cols x 128 positions x 0.4167 ns =
109 us) and the DVE evacuation (~84 us) hide underneath.

Per position n the computation is a complex matmul
  [C=128, B=64]^T @ [C=128, O=512]  (4 real matmuls per position)
with x stationary (M=64) and w moving (O=512 columns). Unlike fp32r,
bf16 matmuls may write PSUM at base partition 64 (PE col-tile position),
so out_re accumulates into partitions 0-63 and out_im into partitions
64-127 of the SAME PSUM bank:
  re: x_re.T@w_re + (-x_im).T@w_im   (-x_im formed on the Pool engine)
  im: x_im.T@w_re +   x_re .T@w_im
A single 128-partition DVE tensor_tensor then adds bias (b_re replicated
on partitions 0-63, b_im on 64-127) and writes the bf16 output tile, so
stores run 128 partitions wide across all 16 SDMA engines.

Host-side prep packs x as (C, N, 2, B) bf16 [re, im] and w as
(C, N, 2, O) bf16 [re, im] so every DMA is long-contiguous per
partition; loads are spread across the SP and ACT HWDGE rings and the
output store is split in halves across both. The kernel writes out as
(128, NSH, O) bf16 per core (partitions 0-63 = re over b, 64-127 = im)
and the host assembles complex64 (B, O, N).

Device work runs in a spawned subprocess (fresh PJRT/axon session per
attempt, up to 4 attempts) because intermittent NRT_EXEC_UNIT errors
poison a session for the life of its process.
"""

import numpy as np

B, C, N, O = 64, 128, 1024, 512
NCORES = 8
NSH = N // NCORES        # 128 positions per core
JT = 8                   # positions per j-tile
NT = NSH // JT           # 16 j-tiles per core


def build_nc(loop_r=None, timing_pool=None, parts="all", jt=None, bufs=(3, 2),
             split_w=True, neg_x=True, store_ring="scalar", split_out=True,
             evac_pool=False, w_chunks=None, tile_major=False, w_h=None):
    """Build the per-core Tile program.

    loop_r: wrap the body in a hardware For_i loop (timing only).
    timing_pool: if set (e.g. 2), DRAM in/out tensors cover only that many
    j-tiles and the body cycles through them — keeps the uploaded bytes tiny
    for loop-delta timing while preserving per-iteration DMA/compute work.
    parts: "all" | "dma" (skip compute) | "noout" (skip output store).
    split_w: issue the w load as two halves on the SP and ACT HWDGE rings.
    neg_x: ship x as [re, im] and form -im on the idle Pool engine (saves
    one DMA plane); else ship [re, im, -im] from the host.
    store_ring: HWDGE ring for the output store ("sync"|"vector"|"scalar").
    split_out: store the output tile as two halves (SP ring + store_ring),
    each gated only on its own half's evacuations.
    evac_pool: evacuate odd positions on the Pool engine (DVE does evens).
    """
    import concourse.mybir as mybir
    from concourse import bacc
    from concourse.tile import TileContext

    bf16 = mybir.dt.bfloat16
    f32 = mybir.dt.float32
    add = mybir.AluOpType.add

    nc = bacc.Bacc(None, target_bir_lowering=False, debug=False)

    jt = JT if jt is None else jt
    nt = NSH // jt
    xpl = 2 if neg_x else 3
    pool_t = nt if timing_pool is None else timing_pool
    pool_n = pool_t * jt
    if tile_major:
        # tile-major DRAM layouts: each j-tile's transfer is one fully
        # contiguous DRAM block (better HBM locality than C-major)
        x_d = nc.dram_tensor("xt", (pool_t, C, jt, xpl, B), bf16,
                             kind="ExternalInput")
        w_d = nc.dram_tensor("wt", (pool_t, C, jt, 2, O), bf16,
                             kind="ExternalInput")
        out_d = nc.dram_tensor("out", (pool_t, 2 * B, jt, O), bf16,
                               kind="ExternalOutput")
    else:
        x_d = nc.dram_tensor("xt", (C, pool_n, xpl, B), bf16,
                             kind="ExternalInput")
        w_d = nc.dram_tensor("wt", (C, pool_n, 2, O), bf16,
                             kind="ExternalInput")
        out_d = nc.dram_tensor("out", (2 * B, pool_n, O), bf16,
                               kind="ExternalOutput")
    b_d = nc.dram_tensor("bt", (2 * B, O), f32, kind="ExternalInput")
    store_eng = {"sync": nc.sync, "scalar": nc.scalar,
                 "gpsimd": nc.gpsimd}[store_ring]

    with TileContext(nc) as tc:
        with (
            tc.tile_pool(name="xw", bufs=bufs[0]) as xw,
            tc.tile_pool(name="ob", bufs=bufs[1]) as ob,
            tc.tile_pool(name="cst", bufs=1) as cst,
            tc.tile_pool(name="ps", bufs=3, space="PSUM") as ps,
        ):
            b_t = cst.tile([2 * B, O], f32)
            nc.sync.dma_start(out=b_t[:], in_=b_d[:])

            def one_position(x_t, xn_t, w_t, o_t, j):
                ps_t = ps.tile([2 * B, O], mybir.dt.float32, name="ps_t")
                x_re = x_t[:, j, 0, :]
                x_im = x_t[:, j, 1, :]
                x_imn = xn_t[:, j, :] if neg_x else x_t[:, j, 2, :]
                w_re = w_t[:, j, 0, :]
                w_im = w_t[:, j, 1, :]
                nc.tensor.matmul(ps_t[0:B, :], x_re, w_re, start=True, stop=False)
                nc.tensor.matmul(ps_t[0:B, :], x_imn, w_im, start=False, stop=True)
                nc.tensor.matmul(ps_t[B:, :], x_im, w_re, start=True, stop=False)
                nc.tensor.matmul(ps_t[B:, :], x_re, w_im, start=False, stop=True)
                eng = nc.gpsimd if (evac_pool and j % 2) else nc.vector
                eng.tensor_tensor(o_t[:, j, :], ps_t[:], b_t[:], add)

            def body(_i=None):
                for jt_i in range(nt):
                    x_t = xw.tile([C, jt, xpl, B], bf16, name="x_t")
                    w_t = xw.tile([C, jt, 2, O], bf16, name="w_t")
                    xn_t = xw.tile([C, jt, B], bf16, name="xn_t") if neg_x else None
                    o_t = ob.tile([2 * B, jt, O], bf16, name="o_t")
                    eff = jt_i if timing_pool is None else jt_i % timing_pool
                    sl = slice(eff * jt, (eff + 1) * jt)
                    if tile_major:
                        x_src = x_d[eff]
                        w_src = w_d[eff]
                        o_dst = out_d[eff]
                    else:
                        x_src = x_d[:, sl]
                        w_src = w_d[:, sl]
                        o_dst = None
                    if w_h:
                        # asymmetric w split (ring balance) + w issued
                        # before x so the big transfers lead each tile
                        nc.sync.dma_start(out=w_t[:, :w_h], in_=w_src[:, :w_h])
                        nc.scalar.dma_start(out=w_t[:, w_h:], in_=w_src[:, w_h:])
                    nc.scalar.dma_start(out=x_t[:], in_=x_src)
                    if neg_x:
                        nc.gpsimd.tensor_scalar_mul(xn_t[:], x_t[:, :, 1, :], -1.0)
                    if w_h:
                        pass
                    elif w_chunks:
                        cs = jt // w_chunks
                        for ci in range(w_chunks):
                            eng = nc.sync if ci % 2 == 0 else nc.scalar
                            wsl = slice(ci * cs, (ci + 1) * cs)
                            eng.dma_start(out=w_t[:, wsl], in_=w_src[:, wsl])
                    elif split_w:
                        h = jt // 2
                        nc.sync.dma_start(out=w_t[:, :h], in_=w_src[:, :h])
                        nc.scalar.dma_start(out=w_t[:, h:], in_=w_src[:, h:])
                    else:
                        nc.sync.dma_start(out=w_t[:], in_=w_src)
                    for j in range(jt) if parts != "dma" else []:
                        one_position(x_t, xn_t, w_t, o_t, j)
                    if parts != "noout":
                        if parts == "dma":
                            nc.vector.memset(o_t[0:1, 0, 0:1], 0.0)
                        h = jt // 2
                        if tile_major and split_out:
                            nc.sync.dma_start(out=o_dst[:, :h], in_=o_t[:, :h])
                            store_eng.dma_start(out=o_dst[:, h:], in_=o_t[:, h:])
                        elif tile_major:
                            store_eng.dma_start(out=o_dst[:], in_=o_t[:])
                        elif split_out:
                            sl_a = slice(sl.start, sl.start + h)
                            sl_b = slice(sl.start + h, sl.stop)
                            nc.sync.dma_start(out=out_d[:, sl_a],
                                              in_=o_t[:, :h])
                            store_eng.dma_start(out=out_d[:, sl_b],
                                                in_=o_t[:, h:])
                        else:
                            store_eng.dma_start(out=out_d[:, sl], in_=o_t[:])

            if loop_r is None:
                body()
            else:
                with tc.For_i(0, loop_r, 1):
                    body()

    nc.compile()
    return nc


def _prep_inputs(x_re, x_im, w_re, w_im, b_re, b_im):
    """Host-side packing/transposition into the kernel's DMA-friendly bf16
    layouts. Threaded over blocks to speed up the big w transpose."""
    from concurrent.futures import ThreadPoolExecutor
    import ml_dtypes

    bf16 = ml_dtypes.bfloat16
    x_re = np.asarray(x_re, dtype=np.float32)
    x_im = np.asarray(x_im, dtype=np.float32)
    w_re = np.asarray(w_re, dtype=np.float32)
    w_im = np.asarray(w_im, dtype=np.float32)
    b_re = np.asarray(b_re, dtype=np.float32)
    b_im = np.asarray(b_im, dtype=np.float32)

    # xt: (C, N, 2, B) <- [x_re, x_im] transposed from (B, C, N)
    # (the kernel forms -x_im on-chip)
    xt = np.empty((C, N, 2, B), bf16)
    # wt: (C, N, 2, O) <- [w_re, w_im] transposed from (O, C, N)
    wt = np.empty((C, N, 2, O), bf16)

    def do_x(k):
        src = x_re if k == 0 else x_im
        xt[:, :, k, :] = src.transpose(1, 2, 0)

    def do_w(args):
        k, c0 = args
        src = w_re[0] if k == 0 else w_im[0]
        # copy block of c rows: dst (cblk, N, O) <- src (O, cblk, N)
        wt[c0:c0 + 16, :, k, :] = src[:, c0:c0 + 16, :].transpose(1, 2, 0)

    with ThreadPoolExecutor(max_workers=16) as ex:
        futs = [ex.submit(do_x, k) for k in range(2)]
        futs += [ex.submit(do_w, (k, c0)) for k in range(2)
                 for c0 in range(0, C, 16)]
        for f in futs:
            f.result()

    # bt: (128, O) f32 — b_re replicated on partitions 0-63, b_im on 64-127
    bt = np.empty((2 * B, O), np.float32)
    bt[:B, :] = b_re[0, :, 0][None, :]
    bt[B:, :] = b_im[0, :, 0][None, :]

    in_maps = []
    for c in range(NCORES):
        sl = slice(c * NSH, (c + 1) * NSH)
        in_maps.append({
            "xt": np.ascontiguousarray(xt[:, sl]),
            "wt": np.ascontiguousarray(wt[:, sl]),
            "bt": bt,
        })
    return in_maps


def _assemble(results):
    """Per-core 'out' buffers (128, NSH, O) bf16 -> (B, O, N) complex64."""
    out = np.empty((B, O, N), np.complex64)
    for c in range(NCORES):
        buf = results[c]["out"]
        # partitions 0-63: re over b; partitions 64-127: im over b
        blk = buf[:B].astype(np.float32) + 1j * buf[B:].astype(np.float32)
        out[:, :, c * NSH:(c + 1) * NSH] = blk.transpose(0, 2, 1)
    return out


def _device_worker(in_path, out_path):
    """Subprocess entry: build + run the kernel on all 8 cores.

    Runs in its own process so every attempt gets a fresh PJRT/axon
    session — intermittent NRT_EXEC_UNIT_UNRECOVERABLE errors poison the
    session for the life of the process, so in-process retries can't
    recover but a fresh process can.
    """
    import os
    import pickle

    os.environ.setdefault("JAX_PLATFORMS", "axon,cpu")
    from concourse import bass_utils

    with open(in_path, "rb") as f:
        in_maps = pickle.load(f)
    nc = build_nc()
    res = bass_utils.run_bass_kernel_spmd(nc, in_maps,
                                          core_ids=list(range(NCORES)))
    with open(out_path, "wb") as f:
        pickle.dump([{"out": np.asarray(r["out"])} for r in res.results], f,
                    protocol=4)


def kernel(x_re, x_im, w_re, w_im, b_re, b_im):
    import os
    import pickle
    import subprocess
    import sys
    import tempfile
    import time

    in_maps = _prep_inputs(x_re, x_im, w_re, w_im, b_re, b_im)
    kdir = os.path.dirname(os.path.abspath(__file__))
    with tempfile.TemporaryDirectory() as td:
        in_path = os.path.join(td, "in.pkl")
        out_path = os.path.join(td, "out.pkl")
        with open(in_path, "wb") as f:
            pickle.dump(in_maps, f, protocol=4)
        code = (f"import sys; sys.path.insert(0, {kdir!r}); "
                f"import kernel; "
                f"kernel._device_worker({in_path!r}, {out_path!r})")
        last_err = None
        for attempt in range(4):
            if attempt:
                time.sleep(10 * attempt)
            r = subprocess.run([sys.executable, "-c", code],
                               capture_output=True, text=True)
            if r.returncode == 0 and os.path.exists(out_path):
                break
            last_err = r.stderr[-2000:] if r.stderr else f"rc={r.returncode}"
        else:
            raise RuntimeError(f"device worker failed 4 times; last: {last_err}")
        with open(out_path, "rb") as f:
            results = pickle.load(f)
    return _assemble(results)
